# revision 12
# baseline (speedup 1.0000x reference)
"""Trainium2 Bass kernel for nn_CriticRNN (GRU critic network).

8 NeuronCores, data-parallel over batch B=512 -> 64 envs/core.
  Phase A: emb = relu(ws @ Wd + bd) + LayerNorm  (rows-on-partitions, fp32r
           matmuls, bn_stats). ln_scale/ln_bias folded into Wi/bi on host.
           embn (bf16) -> DRAM.
  Phase B: xprojT = Wi'.T @ embnT + bi'  (features-on-partitions; embn read
           back via xbar-transposed DMA; bf16 matmuls) -> DRAM, t-major.
  Phase C: sequential GRU scan, features-on-partitions. hproj via bf16
           matmuls (Wh chunks stationary); xp(r,z) injected into PSUM via
           identity-matmuls; gates on ACT + DVE; phase-B tail and the critic
           head (fp32r) spread between scan steps as PE filler tasks.
"""

from collections import deque

import numpy as np
import ml_dtypes

T, B, OBS, H, FC = 128, 512, 1024, 512, 512
LN_EPS = 1e-6
NCORES = 8
BL = B // NCORES            # 64 envs per core
ROWS = T * BL               # 8192 rows per core, row = t*BL + b
H3 = 3 * H
NMC = ROWS // 128           # 64 row-chunks (phase A)
NNC = ROWS // 512           # 16 row-groups of 512 = 8 timesteps (phase B)
NJC = H3 // 128             # 12 xproj chunks
NKC = H // 128              # 4 h chunks
NOC = OBS // 128            # 8 obs chunks
# psum chunk c -> xproj j-chunk: c0-3 = r (j0-3), c4-7 = n (j8-11), c8-11 = z (j4-7)
PS2J = [0, 1, 2, 3, 8, 9, 10, 11, 4, 5, 6, 7]

_cache = {}


def _build():
    import concourse.bacc as bacc
    import concourse.tile as tile
    from concourse import mybir
    from concourse.masks import make_identity

    f32 = mybir.dt.float32
    f32r = mybir.dt.float32r
    bf16 = mybir.dt.bfloat16
    AF = mybir.ActivationFunctionType
    OP = mybir.AluOpType

    nc = bacc.Bacc("TRN2", num_devices=NCORES, debug=False,
                   target_bir_lowering=False)

    d_wsT = nc.dram_tensor("wsT", [OBS, ROWS], f32r, kind="ExternalInput")
    d_Wd = nc.dram_tensor("Wd", [OBS, H], f32r, kind="ExternalInput")
    d_bd = nc.dram_tensor("bd", [1, H], f32r, kind="ExternalInput")
    d_Wip = nc.dram_tensor("Wip", [H, H3], bf16, kind="ExternalInput")
    d_bip = nc.dram_tensor("bip", [1, H3], f32r, kind="ExternalInput")
    d_Whb = nc.dram_tensor("Whb", [H, H3], bf16, kind="ExternalInput")
    d_bhn = nc.dram_tensor("bhn", [1, H], f32r, kind="ExternalInput")
    d_Wc1 = nc.dram_tensor("Wc1", [H, FC], f32r, kind="ExternalInput")
    d_bc1 = nc.dram_tensor("bc1", [FC], f32, kind="ExternalInput")
    d_Wc2 = nc.dram_tensor("Wc2", [FC, 1], f32r, kind="ExternalInput")
    d_bc2 = nc.dram_tensor("bc2", [1, 1], f32r, kind="ExternalInput")
    d_h0T = nc.dram_tensor("h0T", [H, BL], f32, kind="ExternalInput")
    d_nd = nc.dram_tensor("nd", [T, BL], bf16, kind="ExternalInput")  # 1-dones
    d_ones = nc.dram_tensor("ones", [1, 512], f32r, kind="ExternalInput")

    d_hlastT = nc.dram_tensor("hlastT", [NKC, 128, BL], f32,
                              kind="ExternalOutput")
    d_value = nc.dram_tensor("value", [T, BL], f32, kind="ExternalOutput")

    with tile.TileContext(nc) as tc:
        with tc.tile_pool(name="dram", bufs=1, space="DRAM") as dram, \
             tc.tile_pool(name="consts", bufs=1) as consts:
            d_embn = dram.tile([ROWS, H], bf16)
            # [group, p, j_chunk, t_in_group, b]
            d_xp = dram.tile([NNC, 128, NJC, 8, BL], bf16)

            # resident weights/constants
            wd_sb = consts.tile([128, NOC, H], f32r)
            nc.sync.dma_start(out=wd_sb, in_=d_Wd.ap().rearrange(
                "(k p) j -> p k j", p=128))
            bd_sb = consts.tile([1, H], f32r)
            nc.sync.dma_start(out=bd_sb, in_=d_bd.ap())
            wip_sb = consts.tile([128, NKC, H3], bf16)
            nc.sync.dma_start(out=wip_sb, in_=d_Wip.ap().rearrange(
                "(k p) j -> p k j", p=128))
            bip_sb = consts.tile([1, H3], f32r)
            nc.sync.dma_start(out=bip_sb, in_=d_bip.ap())
            whb_sb = consts.tile([128, NKC, H3], bf16)
            nc.sync.dma_start(out=whb_sb, in_=d_Whb.ap().rearrange(
                "(k p) j -> p k j", p=128))
            bhn_sb = consts.tile([1, H], f32r)
            nc.sync.dma_start(out=bhn_sb, in_=d_bhn.ap())
            wc1_sb = consts.tile([128, NKC, FC], f32r)
            nc.sync.dma_start(out=wc1_sb, in_=d_Wc1.ap().rearrange(
                "(k p) j -> p k j", p=128))
            bc1_sb = consts.tile([128, NKC], f32)
            nc.sync.dma_start(out=bc1_sb, in_=d_bc1.ap().rearrange(
                "(k p) -> p k", p=128))
            wc2_sb = consts.tile([128, NKC, 1], f32r)
            nc.sync.dma_start(out=wc2_sb, in_=d_Wc2.ap().rearrange(
                "(k p) j -> p k j", p=128))
            bc2_sb = consts.tile([1, 1], f32r)
            nc.sync.dma_start(out=bc2_sb, in_=d_bc2.ap())
            h0_sb = consts.tile([128, NKC, BL], f32)
            nc.sync.dma_start(out=h0_sb, in_=d_h0T.ap().rearrange(
                "(k p) b -> p k b", p=128))
            nd_sb = consts.tile([128, T, BL], bf16)
            nc.sync.dma_start(out=nd_sb, in_=d_nd.ap().unsqueeze(0)
                              .broadcast_to([128, T, BL]))
            ident = consts.tile([128, 128], bf16)
            make_identity(nc, ident[:])
            ones_r = consts.tile([1, 512], f32r)
            nc.sync.dma_start(out=ones_r, in_=d_ones.ap())
            eps_sb = consts.tile([128, 1], f32)
            nc.vector.memset(eps_sb[:], LN_EPS)

            # ================= Phase A: embed + LN =================
            with tc.tile_pool(name="pa", bufs=3) as pa, \
                 tc.tile_pool(name="pa_st", bufs=4) as pa_st, \
                 tc.tile_pool(name="pa_ps", bufs=2, space="PSUM") as pa_ps:
                for m in range(NMC):
                    rs = slice(128 * m, 128 * (m + 1))
                    ws_m = pa.tile([128, NOC, 128], f32r, tag="ws")
                    nc.sync.dma_start(out=ws_m, in_=d_wsT.ap()[:, rs]
                                      .rearrange("(k p) r -> p k r", p=128))
                    ps = pa_ps.tile([128, H], f32, tag="aps")
                    for kc in range(NOC):
                        nc.tensor.matmul(ps[:], ws_m[:, kc, :], wd_sb[:, kc, :],
                                         start=(kc == 0), stop=False)
                    nc.tensor.matmul(ps[:], ones_r[:, :128], bd_sb[:],
                                     start=False, stop=True)
                    emb = pa.tile([128, H], f32, tag="emb")
                    nc.scalar.activation(emb[:], ps[:], AF.Relu)
                    st6 = pa_st.tile([128, 6], f32, tag="st6")
                    nc.vector.bn_stats(out=st6[:], in_=emb[:])
                    mv = pa_st.tile([128, 2], f32, tag="mv")
                    nc.vector.bn_aggr(out=mv[:], in_=st6[:])
                    std = pa_st.tile([128, 1], f32, tag="std")
                    nc.scalar.activation(std[:], mv[:, 1:2], AF.Sqrt,
                                         bias=eps_sb[:])
                    rstd = pa_st.tile([128, 1], f32, tag="rstd")
                    nc.vector.reciprocal(out=rstd[:], in_=std[:])
                    ebn = pa.tile([128, H], bf16, tag="ebn")
                    nc.vector.tensor_scalar(out=ebn[:], in0=emb[:],
                                            scalar1=mv[:, 0:1],
                                            scalar2=rstd[:],
                                            op0=OP.subtract, op1=OP.mult)
                    nc.sync.dma_start(out=d_embn[rs, :], in_=ebn[:])

            # ============ Phase C (+ phase B / critic as fillers) ============
            with tc.tile_pool(name="pb", bufs=2) as pb, \
                 tc.tile_pool(name="pb_ps", bufs=1, space="PSUM") as pb_ps, \
                 tc.tile_pool(name="pc", bufs=3) as pc, \
                 tc.tile_pool(name="pxp", bufs=6) as pxp, \
                 tc.tile_pool(name="pys", bufs=2) as pys, \
                 tc.tile_pool(name="pcr", bufs=2) as pcr, \
                 tc.tile_pool(name="ps_s", bufs=2, space="PSUM") as ps_s, \
                 tc.tile_pool(name="ps_c", bufs=2, space="PSUM") as ps_c, \
                 tc.tile_pool(name="ps_v", bufs=1, space="PSUM") as ps_v:

                ebT_cur = {}

                def task_b_load(n):
                    def _f():
                        rs = slice(512 * n, 512 * (n + 1))
                        tiles = []
                        for kc in range(NKC):
                            t_ = pb.tile([128, 512], bf16, tag=f"ebT{kc}")
                            nc.sync.dma_start_transpose(
                                out=t_[:],
                                in_=d_embn[rs, 128 * kc:128 * (kc + 1)])
                            tiles.append(t_)
                        ebT_cur[n] = tiles
                    return _f

                def task_b_mm(n, mj):
                    def _f():
                        ebT = ebT_cur[n]
                        js = slice(128 * mj, 128 * (mj + 1))
                        ps = pb_ps.tile([128, 512], f32, tag="xps")
                        for kc in range(NKC):
                            nc.tensor.matmul(ps[:], wip_sb[:, kc, js],
                                             ebT[kc][:],
                                             start=(kc == 0), stop=False)
                        nc.tensor.matmul(ps[:], bip_sb[:, js], ones_r[:],
                                         start=False, stop=True)
                        xps = pb.tile([128, 8, BL], bf16, tag="xps_sb")
                        src = ps[:].rearrange("p (t b) -> p t b", b=BL)
                        if mj % 2 == 0:
                            nc.scalar.activation(xps[:], src, AF.Copy)
                        else:
                            nc.vector.tensor_copy(xps[:], src)
                        nc.sync.dma_start(out=d_xp[n, :, mj, :, :], in_=xps[:])
                        if mj == NJC - 1:
                            del ebT_cur[n]
                    return _f

                cr_cur = {}

                def task_critic_mf(ys_tile, n, mf):
                    def _f():
                        cps = ps_c.tile([128, 512], f32, tag="cps")
                        for kc in range(NKC):
                            nc.tensor.matmul(
                                cps[:],
                                wc1_sb[:, kc, 128 * mf:128 * (mf + 1)],
                                ys_tile[:, :, kc, :],
                                start=(kc == 0), stop=(kc == 3))
                        cr = pcr.tile([128, 512], f32r, tag=f"cr{mf}")
                        nc.scalar.activation(cr[:], cps[:], AF.Relu,
                                             bias=bc1_sb[:, mf:mf + 1])
                        cr_cur.setdefault(n, [None] * NKC)[mf] = cr
                    return _f

                def task_value(n):
                    def _f():
                        crs = cr_cur.pop(n)
                        vps = ps_v.tile([1, 512], f32, tag="vps")
                        for kc in range(NKC):
                            nc.tensor.matmul(vps[:], wc2_sb[:, kc, :],
                                             crs[kc][:], start=(kc == 0),
                                             stop=False)
                        nc.tensor.matmul(vps[:], bc2_sb[:], ones_r[:],
                                         start=False, stop=True)
                        val = pc.tile([1, 8, BL], f32, tag="val")
                        nc.scalar.activation(
                            val[:], vps[:].rearrange("p (t b) -> p t b", b=BL),
                            AF.Copy)
                        nc.sync.dma_start(
                            out=d_value.ap()[8 * n:8 * (n + 1), :],
                            in_=val[:])
                    return _f

                fillers = deque()
                # prologue: phase B groups 0 and 1 fully before the scan
                for n in range(2):
                    task_b_load(n)()
                    for mj in range(NJC):
                        task_b_mm(n, mj)()
                for n in range(2, 3):
                    fillers.append(task_b_load(n))
                    for mj in range(NJC):
                        fillers.append(task_b_mm(n, mj))

                ys_cur = None
                ys_prev = None
                for t in range(T):
                    n, ti = divmod(t, 8)
                    if ti == 0:
                        ys_prev = ys_cur
                        ys_cur = pys.tile([128, 8, NKC, BL], f32r, tag="ys")
                        nb = n + 3
                        if nb < NNC:
                            fillers.append(task_b_load(nb))
                            for mj in range(NJC):
                                fillers.append(task_b_mm(nb, mj))

                    xpt = pxp.tile([128, NJC, BL], bf16, tag="xpt")
                    nc.sync.dma_start(out=xpt, in_=d_xp[n, :, :, ti, :])

                    if t == 0:
                        h_prev = h0_sb[:]
                    elif ti == 0:
                        h_prev = ys_prev[:, 7, :, :]
                    else:
                        h_prev = ys_cur[:, ti - 1, :, :]
                    heff = pc.tile([128, NKC, BL], bf16, tag="heff")
                    nc.vector.tensor_tensor(
                        out=heff[:], in0=h_prev,
                        in1=nd_sb[:, t:t + 1, :].broadcast_to([128, NKC, BL]),
                        op=OP.mult)

                    # hproj: psum chunks c0-3 = r, c4-7 = n, c8-11 = z
                    ps = ps_s.tile([128, NJC, BL], f32, tag="sps")
                    for c in range(NJC):
                        j = PS2J[c]
                        js = slice(128 * j, 128 * (j + 1))
                        if c < 4 or c >= 8:      # r / z: inject xp
                            nc.tensor.matmul(ps[:, c, :], ident[:],
                                             xpt[:, j, :],
                                             start=True, stop=False)
                            for kc in range(NKC):
                                nc.tensor.matmul(
                                    ps[:, c, :], whb_sb[:, kc, js],
                                    heff[:, kc, :],
                                    start=False, stop=(kc == 3))
                        else:                    # n: hproj + bhn
                            for kc in range(NKC):
                                nc.tensor.matmul(
                                    ps[:, c, :], whb_sb[:, kc, js],
                                    heff[:, kc, :],
                                    start=(kc == 0), stop=False)
                            nc.tensor.matmul(
                                ps[:, c, :],
                                bhn_sb[:, 128 * (c - 4):128 * (c - 3)],
                                ones_r[:, :BL], start=False, stop=True)

                    r_sb = pc.tile([128, NKC, BL], f32, tag="r_sb")
                    nc.scalar.activation(r_sb[:], ps[:, 0:4, :], AF.Sigmoid)
                    rn = pc.tile([128, NKC, BL], f32, tag="rn")
                    nc.vector.tensor_tensor(out=rn[:], in0=ps[:, 4:8, :],
                                            in1=r_sb[:], op=OP.mult)
                    npre = pc.tile([128, NKC, BL], f32, tag="npre")
                    nc.vector.tensor_tensor(out=npre[:], in0=rn[:],
                                            in1=xpt[:, 8:12, :], op=OP.add)
                    nt = pc.tile([128, NKC, BL], f32, tag="nt")
                    nc.scalar.activation(nt[:], npre[:], AF.Tanh)
                    dd = pc.tile([128, NKC, BL], f32, tag="dd")
                    nc.vector.tensor_tensor(out=dd[:], in0=heff[:], in1=nt[:],
                                            op=OP.subtract)
                    z_sb = pc.tile([128, NKC, BL], f32, tag="z_sb")
                    nc.scalar.activation(z_sb[:], ps[:, 8:12, :], AF.Sigmoid)
                    zd = pc.tile([128, NKC, BL], f32, tag="zd")
                    nc.vector.tensor_tensor(out=zd[:], in0=z_sb[:], in1=dd[:],
                                            op=OP.mult)
                    nc.vector.tensor_tensor(out=ys_cur[:, ti, :, :],
                                            in0=nt[:], in1=zd[:], op=OP.add)

                    if ti == 7:
                        for mf in range(NKC):
                            fillers.append(task_critic_mf(ys_cur, n, mf))
                        fillers.append(task_value(n))

                    budget = 2 if len(fillers) < 14 else 3
                    for _ in range(budget):
                        if fillers:
                            fillers.popleft()()

                while fillers:
                    fillers.popleft()()

                nc.sync.dma_start(
                    out=d_hlastT.ap().rearrange("k p b -> p k b"),
                    in_=ys_cur[:, 7, :, :].bitcast(f32))

    nc.compile()
    return nc


def _prep_inputs(hidden, world_state, dones, Wd, bd, ln_scale, ln_bias,
                 Wi, bi, Wh, bhn, Wc1, bc1, Wc2, bc2):
    """Host-side shard + layout prep. Returns in_maps (list of 8 dicts)."""
    f32 = np.float32
    bf = ml_dtypes.bfloat16
    hidden = np.asarray(hidden, f32)
    world_state = np.asarray(world_state, f32)
    Wd = np.asarray(Wd, f32)
    bd = np.asarray(bd, f32).reshape(1, H)
    Wip = (np.asarray(ln_scale, f32)[:, None] * np.asarray(Wi, f32)).astype(bf)
    bip = (np.asarray(bi, f32) +
           np.asarray(ln_bias, f32) @ np.asarray(Wi, f32)).reshape(1, H3)
    Whb = np.asarray(Wh, f32).astype(bf)
    bhn = np.asarray(bhn, f32).reshape(1, H)
    Wc1 = np.asarray(Wc1, f32)
    bc1 = np.asarray(bc1, f32)
    Wc2 = np.asarray(Wc2, f32)
    bc2 = np.asarray(bc2, f32).reshape(1, 1)
    nd_full = (1.0 - np.asarray(dones, f32)).astype(bf)     # [T, B]

    in_maps = []
    for c in range(NCORES):
        sl = slice(BL * c, BL * (c + 1))
        ws_c = world_state[:, sl, :]                        # [T, BL, OBS]
        wsT = np.ascontiguousarray(
            ws_c.transpose(2, 0, 1).reshape(OBS, ROWS)).astype(f32)
        in_maps.append(dict(
            wsT=wsT, Wd=Wd, bd=bd, Wip=np.ascontiguousarray(Wip),
            bip=bip.astype(f32), Whb=np.ascontiguousarray(Whb),
            bhn=bhn, Wc1=Wc1, bc1=bc1, Wc2=Wc2, bc2=bc2,
            h0T=np.ascontiguousarray(hidden[sl].T),
            nd=np.ascontiguousarray(nd_full[:, sl]),
            ones=np.ones((1, 512), f32),
        ))
    return in_maps


def kernel(**inputs):
    from concourse.bass_utils import run_bass_kernel_spmd

    if "nc" not in _cache:
        _cache["nc"] = _build()
    nc = _cache["nc"]
    in_maps = _prep_inputs(**inputs)
    res = run_bass_kernel_spmd(nc, in_maps, core_ids=list(range(NCORES)),
                               **_cache.get("run_kwargs", {}))
    _cache["last_result"] = res
    h_last = np.concatenate(
        [np.asarray(r["hlastT"], dtype=np.float32).reshape(H, BL).T
         for r in res.results], axis=0)
    value = np.concatenate(
        [np.asarray(r["value"], dtype=np.float32) for r in res.results],
        axis=1)
    return h_last, value


# revision 24
# speedup vs baseline: 1.1521x; 1.1521x over previous
"""Trainium2 Bass kernel for nn_CriticRNN (GRU critic network).

8 NeuronCores, data-parallel over batch B=512 -> 64 envs/core.
  Phase A: emb = relu(ws @ Wd + bd) + LayerNorm  (rows-on-partitions, fp32r
           matmuls, bn_stats). ln_scale/ln_bias folded into Wi/bi on host.
           embn (bf16) -> DRAM.
  Phase B: xprojT = Wi'.T @ embnT + bi'  (features-on-partitions; embn read
           back via xbar-transposed DMA; bf16 matmuls) -> DRAM, t-major.
  Phase C: sequential GRU scan, features-on-partitions. hproj via bf16
           matmuls (Wh chunks stationary); xp(r,z) injected into PSUM via
           identity-matmuls; gates on ACT + DVE; phase-B tail and the critic
           head (fp32r) spread between scan steps as PE filler tasks.
"""

from collections import deque

import numpy as np
import ml_dtypes

T, B, OBS, H, FC = 128, 512, 1024, 512, 512
LN_EPS = 1e-6
NCORES = 8
BL = B // NCORES            # 64 envs per core
ROWS = T * BL               # 8192 rows per core, row = t*BL + b
H3 = 3 * H
NMC = ROWS // 128           # 64 row-chunks (phase A)
NNC = ROWS // 512           # 16 row-groups of 512 = 8 timesteps (phase B)
NJC = H3 // 128             # 12 xproj chunks
NKC = H // 128              # 4 h chunks
NOC = OBS // 128            # 8 obs chunks
# psum chunks in natural j order: c0-3 = r, c4-7 = z (bank 0), c8-11 = n (bank 1)

_cache = {}


def _build():
    import concourse.bacc as bacc
    import concourse.tile as tile
    from concourse import mybir
    from concourse.masks import make_identity

    f32 = mybir.dt.float32
    f32r = mybir.dt.float32r
    bf16 = mybir.dt.bfloat16
    AF = mybir.ActivationFunctionType
    OP = mybir.AluOpType

    nc = bacc.Bacc("TRN2", num_devices=NCORES, debug=False,
                   target_bir_lowering=False)

    d_wsT = nc.dram_tensor("wsT", [OBS, ROWS], f32r, kind="ExternalInput")
    d_Wd = nc.dram_tensor("Wd", [OBS, H], f32r, kind="ExternalInput")
    d_bd = nc.dram_tensor("bd", [1, H], bf16, kind="ExternalInput")
    d_Wip = nc.dram_tensor("Wip", [H, H3], bf16, kind="ExternalInput")
    d_bip = nc.dram_tensor("bip", [H3], f32, kind="ExternalInput")
    d_Whb = nc.dram_tensor("Whb", [H, H3], bf16, kind="ExternalInput")
    d_bhn = nc.dram_tensor("bhn", [NKC, 128, BL], bf16,
                           kind="ExternalInput")
    d_Wc1 = nc.dram_tensor("Wc1", [H, FC], f32r, kind="ExternalInput")
    d_bc1 = nc.dram_tensor("bc1", [FC], f32, kind="ExternalInput")
    d_Wc2 = nc.dram_tensor("Wc2", [FC, 1], f32r, kind="ExternalInput")
    d_bc2 = nc.dram_tensor("bc2", [1, 1], f32, kind="ExternalInput")
    d_h0T = nc.dram_tensor("h0T", [H, BL], f32, kind="ExternalInput")
    d_nd = nc.dram_tensor("nd", [T, NKC, BL], bf16,
                          kind="ExternalInput")  # 1-dones, k-replicated

    d_hlastT = nc.dram_tensor("hlastT", [NKC, 128, BL], f32,
                              kind="ExternalOutput")
    d_value = nc.dram_tensor("value", [T, BL], f32, kind="ExternalOutput")

    with tile.TileContext(nc) as tc:
        with tc.tile_pool(name="dram", bufs=1, space="DRAM") as dram, \
             tc.tile_pool(name="consts", bufs=1) as consts:
            d_embn = dram.tile([ROWS, H], bf16)
            # [group, p, j_chunk, t_in_group, b]
            d_xp = dram.tile([NNC, 128, NJC, 8, BL], bf16)

            # resident weights/constants
            wd_sb = consts.tile([128, NOC, H], f32r)
            nc.sync.dma_start(out=wd_sb, in_=d_Wd.ap().rearrange(
                "(k p) j -> p k j", p=128))
            bd_bc = consts.tile([128, H], bf16)
            nc.sync.dma_start(out=bd_bc, in_=d_bd.ap().broadcast_to([128, H]))
            wip_sb = consts.tile([128, NKC, H3], bf16)
            nc.sync.dma_start(out=wip_sb, in_=d_Wip.ap().rearrange(
                "(k p) j -> p k j", p=128))
            bip_sb = consts.tile([128, NJC], f32)
            nc.sync.dma_start(out=bip_sb, in_=d_bip.ap().rearrange(
                "(c p) -> p c", p=128))
            whb_sb = consts.tile([128, NKC, H3], bf16)
            nc.sync.dma_start(out=whb_sb, in_=d_Whb.ap().rearrange(
                "(k p) j -> p k j", p=128))
            bhn_bc = consts.tile([128, NKC, BL], bf16)
            nc.sync.dma_start(out=bhn_bc,
                              in_=d_bhn.ap().rearrange("k p b -> p k b"))
            wc1_sb = consts.tile([128, NKC, FC], f32r)
            nc.sync.dma_start(out=wc1_sb, in_=d_Wc1.ap().rearrange(
                "(k p) j -> p k j", p=128))
            bc1_sb = consts.tile([128, NKC], f32)
            nc.sync.dma_start(out=bc1_sb, in_=d_bc1.ap().rearrange(
                "(k p) -> p k", p=128))
            wc2_sb = consts.tile([128, NKC, 1], f32r)
            nc.sync.dma_start(out=wc2_sb, in_=d_Wc2.ap().rearrange(
                "(k p) j -> p k j", p=128))
            bc2_sb = consts.tile([1, 1], f32)
            nc.sync.dma_start(out=bc2_sb, in_=d_bc2.ap())
            h0_sb = consts.tile([128, NKC, BL], f32)
            nc.sync.dma_start(out=h0_sb, in_=d_h0T.ap().rearrange(
                "(k p) b -> p k b", p=128))
            ident = consts.tile([128, 128], bf16)
            make_identity(nc, ident[:])
            eps_sb = consts.tile([128, 1], f32)
            nc.vector.memset(eps_sb[:], LN_EPS)

            # ================= Phase A: embed + LN =================
            with tc.tile_pool(name="pa", bufs=3) as pa, \
                 tc.tile_pool(name="pa_st", bufs=4) as pa_st, \
                 tc.tile_pool(name="pa_ps", bufs=2, space="PSUM") as pa_ps:
                for m in range(NMC):
                    rs = slice(128 * m, 128 * (m + 1))
                    ws_m = pa.tile([128, NOC, 128], f32r, tag="ws")
                    nc.sync.dma_start(out=ws_m, in_=d_wsT.ap()[:, rs]
                                      .rearrange("(k p) r -> p k r", p=128))
                    ps = pa_ps.tile([128, H], f32, tag="aps")
                    nc.tensor.matmul(ps[:], ident[:], bd_bc[:],
                                     start=True, stop=False)
                    for kc in range(NOC):
                        nc.tensor.matmul(ps[:], ws_m[:, kc, :], wd_sb[:, kc, :],
                                         start=False, stop=(kc == NOC - 1))
                    emb = pa.tile([128, H], f32, tag="emb")
                    nc.scalar.activation(emb[:], ps[:], AF.Relu)
                    st6 = pa_st.tile([128, 6], f32, tag="st6")
                    nc.vector.bn_stats(out=st6[:], in_=emb[:])
                    mv = pa_st.tile([128, 2], f32, tag="mv")
                    nc.vector.bn_aggr(out=mv[:], in_=st6[:])
                    std = pa_st.tile([128, 1], f32, tag="std")
                    nc.scalar.activation(std[:], mv[:, 1:2], AF.Sqrt,
                                         bias=eps_sb[:])
                    rstd = pa_st.tile([128, 1], f32, tag="rstd")
                    nc.vector.reciprocal(out=rstd[:], in_=std[:])
                    ebn = pa.tile([128, H], bf16, tag="ebn")
                    nc.vector.tensor_scalar(out=ebn[:], in0=emb[:],
                                            scalar1=mv[:, 0:1],
                                            scalar2=rstd[:],
                                            op0=OP.subtract, op1=OP.mult)
                    nc.sync.dma_start(out=d_embn[rs, :], in_=ebn[:])

            # ============ Phase C (+ phase B / critic as fillers) ============
            with tc.tile_pool(name="pb", bufs=2) as pb, \
                 tc.tile_pool(name="pb_ps", bufs=1, space="PSUM") as pb_ps, \
                 tc.tile_pool(name="pc", bufs=3) as pc, \
                 tc.tile_pool(name="pxp", bufs=2) as pxp, \
                 tc.tile_pool(name="pys", bufs=2) as pys, \
                 tc.tile_pool(name="pcr", bufs=2) as pcr, \
                 tc.tile_pool(name="ps_s", bufs=2, space="PSUM") as ps_s, \
                 tc.tile_pool(name="ps_c", bufs=2, space="PSUM") as ps_c, \
                 tc.tile_pool(name="ps_v", bufs=1, space="PSUM") as ps_v:

                ebT_cur = {}

                def task_b_load(n):
                    def _f():
                        rs = slice(512 * n, 512 * (n + 1))
                        tiles = []
                        for kc in range(NKC):
                            t_ = pb.tile([128, 512], bf16, tag=f"ebT{kc}")
                            nc.sync.dma_start_transpose(
                                out=t_[:],
                                in_=d_embn[rs, 128 * kc:128 * (kc + 1)])
                            tiles.append(t_)
                        ebT_cur[n] = tiles
                    return _f

                def task_b_mm(n, mj):
                    def _f():
                        ebT = ebT_cur[n]
                        js = slice(128 * mj, 128 * (mj + 1))
                        ps = pb_ps.tile([128, 512], f32, tag="xps")
                        for kc in range(NKC):
                            nc.tensor.matmul(ps[:], wip_sb[:, kc, js],
                                             ebT[kc][:],
                                             start=(kc == 0), stop=(kc == 3))
                        xps = pb.tile([128, 8, BL], bf16, tag="xps_sb")
                        src = ps[:].rearrange("p (t b) -> p t b", b=BL)
                        if mj % 2 == 0:
                            nc.scalar.add(xps[:], src, bip_sb[:, mj:mj + 1])
                        else:
                            nc.vector.tensor_scalar_add(xps[:], src,
                                                        bip_sb[:, mj:mj + 1])
                        nc.sync.dma_start(out=d_xp[n, :, mj, :, :], in_=xps[:])
                        if mj == NJC - 1:
                            del ebT_cur[n]
                    return _f

                cr_cur = {}

                def task_critic_mf(ys_tile, n, mf):
                    def _f():
                        cps = ps_c.tile([128, 512], f32, tag="cps")
                        for kc in range(NKC):
                            nc.tensor.matmul(
                                cps[:],
                                wc1_sb[:, kc, 128 * mf:128 * (mf + 1)],
                                ys_tile[:, :, kc, :],
                                start=(kc == 0), stop=(kc == 3))
                        cr = pcr.tile([128, 512], f32r, tag=f"cr{mf}")
                        nc.scalar.activation(cr[:], cps[:], AF.Relu,
                                             bias=bc1_sb[:, mf:mf + 1])
                        cr_cur.setdefault(n, [None] * NKC)[mf] = cr
                    return _f

                def task_value(n):
                    def _f():
                        crs = cr_cur.pop(n)
                        vps = ps_v.tile([1, 512], f32, tag="vps")
                        for kc in range(NKC):
                            nc.tensor.matmul(vps[:], wc2_sb[:, kc, :],
                                             crs[kc][:], start=(kc == 0),
                                             stop=(kc == 3))
                        val = pc.tile([1, 8, BL], f32, tag="val")
                        nc.scalar.add(
                            val[:], vps[:].rearrange("p (t b) -> p t b", b=BL),
                            bc2_sb[:])
                        nc.sync.dma_start(
                            out=d_value.ap()[8 * n:8 * (n + 1), :],
                            in_=val[:])
                    return _f

                fillers = deque()
                # prologue: phase B groups 0 and 1 fully before the scan
                for n in range(2):
                    task_b_load(n)()
                    for mj in range(NJC):
                        task_b_mm(n, mj)()
                for n in range(2, 3):
                    fillers.append(task_b_load(n))
                    for mj in range(NJC):
                        fillers.append(task_b_mm(n, mj))

                ys_cur = None
                ys_prev = None
                for t in range(T):
                    n, ti = divmod(t, 8)
                    if ti == 0:
                        ys_prev = ys_cur
                        ys_cur = pys.tile([128, 8, NKC, BL], f32r, tag="ys")
                        nb = n + 3
                        if nb < NNC:
                            fillers.append(task_b_load(nb))
                            for mj in range(NJC):
                                fillers.append(task_b_mm(nb, mj))
                        xpg = pxp.tile([128, NJC, 8, BL], bf16, tag="xpg")
                        nc.sync.dma_start(out=xpg, in_=d_xp[n])
                        xpg_cur = xpg
                        ndg = pxp.tile([128, 8, NKC, BL], bf16, tag="ndg")
                        nc.sync.dma_start(
                            out=ndg, in_=d_nd.ap()[8 * n:8 * (n + 1)]
                            .unsqueeze(0).broadcast_to([128, 8, NKC, BL]))
                        ndg_cur = ndg

                    if t == 0:
                        h_prev = h0_sb[:]
                    elif ti == 0:
                        h_prev = ys_prev[:, 7, :, :]
                    else:
                        h_prev = ys_cur[:, ti - 1, :, :]
                    heff = pc.tile([128, NKC, BL], bf16, tag="heff")
                    nc.vector.tensor_tensor(
                        out=heff[:], in0=h_prev,
                        in1=ndg_cur[:, ti, :, :], op=OP.mult)

                    # hproj: psum chunks c0-3 = r, c4-7 = z, c8-11 = n
                    ps = ps_s.tile([128, NJC, BL], f32, tag="sps")
                    for c in range(NJC):
                        js = slice(128 * c, 128 * (c + 1))
                        inj = (xpg_cur[:, c, ti, :] if c < 8
                               else bhn_bc[:, c - 8, :])
                        nc.tensor.matmul(ps[:, c, :], ident[:], inj,
                                         start=True, stop=False)
                        for kc in range(NKC):
                            nc.tensor.matmul(
                                ps[:, c, :], whb_sb[:, kc, js],
                                heff[:, kc, :],
                                start=False, stop=(kc == 3))

                    rz = pc.tile([128, 8, BL], f32, tag="rz")
                    nc.scalar.activation(rz[:], ps[:, 0:8, :], AF.Sigmoid)
                    zc = pc.tile([128, NKC, BL], f32, tag="zc")
                    nc.scalar.activation(zc[:], ps[:, 4:8, :], AF.Sigmoid,
                                         scale=-1.0)
                    rn = pc.tile([128, NKC, BL], f32, tag="rn")
                    nc.vector.tensor_tensor(out=rn[:], in0=ps[:, 8:12, :],
                                            in1=rz[:, 0:4, :], op=OP.mult)
                    npre = pc.tile([128, NKC, BL], f32, tag="npre")
                    nc.vector.tensor_tensor(out=npre[:], in0=rn[:],
                                            in1=xpg_cur[:, 8:12, ti, :],
                                            op=OP.add)
                    zh = pc.tile([128, NKC, BL], f32, tag="zh")
                    nc.vector.tensor_tensor(out=zh[:], in0=rz[:, 4:8, :],
                                            in1=heff[:], op=OP.mult)
                    nt = pc.tile([128, NKC, BL], f32, tag="nt")
                    nc.scalar.activation(nt[:], npre[:], AF.Tanh)
                    zcn = pc.tile([128, NKC, BL], f32, tag="zcn")
                    nc.vector.tensor_tensor(out=zcn[:], in0=zc[:], in1=nt[:],
                                            op=OP.mult)
                    nc.vector.tensor_tensor(out=ys_cur[:, ti, :, :],
                                            in0=zcn[:], in1=zh[:], op=OP.add)

                    if ti == 7:
                        for mf in range(NKC):
                            fillers.append(task_critic_mf(ys_cur, n, mf))
                        fillers.append(task_value(n))

                    budget = 2 if len(fillers) < 14 else 3
                    for _ in range(budget):
                        if fillers:
                            fillers.popleft()()

                while fillers:
                    fillers.popleft()()

                nc.sync.dma_start(
                    out=d_hlastT.ap().rearrange("k p b -> p k b"),
                    in_=ys_cur[:, 7, :, :].bitcast(f32))

    nc.compile()
    return nc


def _prep_inputs(hidden, world_state, dones, Wd, bd, ln_scale, ln_bias,
                 Wi, bi, Wh, bhn, Wc1, bc1, Wc2, bc2):
    """Host-side shard + layout prep. Returns in_maps (list of 8 dicts)."""
    f32 = np.float32
    bf = ml_dtypes.bfloat16
    hidden = np.asarray(hidden, f32)
    world_state = np.asarray(world_state, f32)
    Wd = np.asarray(Wd, f32)
    bd = np.asarray(bd, f32).reshape(1, H).astype(bf)
    Wip = (np.asarray(ln_scale, f32)[:, None] * np.asarray(Wi, f32)).astype(bf)
    bip = (np.asarray(bi, f32) +
           np.asarray(ln_bias, f32) @ np.asarray(Wi, f32)).reshape(H3)
    Whb = np.asarray(Wh, f32).astype(bf)
    bhn = np.ascontiguousarray(np.broadcast_to(
        np.asarray(bhn, f32).reshape(NKC, 128, 1), (NKC, 128, BL))).astype(bf)
    Wc1 = np.asarray(Wc1, f32)
    bc1 = np.asarray(bc1, f32)
    Wc2 = np.asarray(Wc2, f32)
    bc2 = np.asarray(bc2, f32).reshape(1, 1)
    nd_full = (1.0 - np.asarray(dones, f32)).astype(bf)     # [T, B]

    in_maps = []
    for c in range(NCORES):
        sl = slice(BL * c, BL * (c + 1))
        ws_c = world_state[:, sl, :]                        # [T, BL, OBS]
        wsT = np.ascontiguousarray(
            ws_c.transpose(2, 0, 1).reshape(OBS, ROWS)).astype(f32)
        in_maps.append(dict(
            wsT=wsT, Wd=Wd, bd=bd, Wip=np.ascontiguousarray(Wip),
            bip=bip.astype(f32), Whb=np.ascontiguousarray(Whb),
            bhn=bhn, Wc1=Wc1, bc1=bc1, Wc2=Wc2, bc2=bc2,
            h0T=np.ascontiguousarray(hidden[sl].T),
            nd=np.ascontiguousarray(np.broadcast_to(
                nd_full[:, sl][:, None, :], (T, NKC, BL))),
        ))
    return in_maps


def kernel(**inputs):
    from concourse.bass_utils import run_bass_kernel_spmd

    if "nc" not in _cache:
        _cache["nc"] = _build()
    nc = _cache["nc"]
    in_maps = _prep_inputs(**inputs)
    res = run_bass_kernel_spmd(nc, in_maps, core_ids=list(range(NCORES)),
                               **_cache.get("run_kwargs", {}))
    _cache["last_result"] = res
    h_last = np.concatenate(
        [np.asarray(r["hlastT"], dtype=np.float32).reshape(H, BL).T
         for r in res.results], axis=0)
    value = np.concatenate(
        [np.asarray(r["value"], dtype=np.float32) for r in res.results],
        axis=1)
    return h_last, value
